# revision 3
# baseline (speedup 1.0000x reference)
"""Trainium2 Bass kernel for CachingMultiHeadAttention (GQA + RoPE + softcap).

Reference semantics (B=2, S=2048, D=4096, 32 q-heads, 8 kv-heads, hd=128):
    qh = rope(x_q @ Wq); kh = rope(x_k @ Wk); vh = x_v @ Wv
    logits = softcap_30(qh kh^T / sqrt(128)) causal-masked
    out = softmax(logits) vh @ Wo ; also returns cache_k, cache_v

Sharding: 8 cores = (2 batches) x (4 kv-head pairs). Each core computes its
batch's projections for its 2 kv-heads / 8 q-heads against full D_MODEL,
attention, and a partial out^T (summed on host over the 4 cores per batch).

All device matmuls run in float32r (fast fp32 mode, ~1 cyc/row at N>=256).
Activations are fed pre-transposed ([D, S]) from the host so every matmul
contracts over the partition dim without on-device transposes.
"""
import sys
sys.path.insert(0, "/opt/trn_rl_repo")

import numpy as np

import concourse.bass as bass
from concourse import bacc
import concourse.mybir as mybir
import concourse.tile as tile
import concourse.alu_op_type as alu
from concourse.bass_utils import run_bass_kernel_spmd

F32 = mybir.dt.float32
F32R = mybir.dt.float32r
AF = mybir.ActivationFunctionType

# Problem constants
B, S, DM = 2, 2048, 4096
NQ, NKV, HD = 32, 8, 128
GROUP = NQ // NKV           # 4 q-heads per kv-head
ATTN_MULT = 1.0 / np.sqrt(128.0)
MAX_ATTN = 30.0
ROPE_BASE = 10000.0

P = 128                     # partitions
NCORES = 8
HEADS_PER_CORE = NQ // (NCORES // B)      # 8 q-heads per core
KV_PER_CORE = NKV // (NCORES // B)        # 2 kv-heads per core
DQ = HEADS_PER_CORE * HD                  # 1024 projected q dims per core
DKV = KV_PER_CORE * HD                    # 256 projected k/v dims per core


def _rope_tables(s):
    """cos table and sign-baked sin table, [HD, s] f32.

    rope(x)[d,t] = x[d,t]*cos[d,t] + x[(d+64)%128, t]*sin_sgn[d,t]
    with sin_sgn negative for d < 64 (matches -x2 in the reference).
    """
    j = np.arange(HD // 2, dtype=np.float64)
    inv_freq = ROPE_BASE ** (-2.0 * j / HD)
    t = np.arange(s, dtype=np.float64)
    phase = np.concatenate([inv_freq, inv_freq])[:, None] * t[None, :]  # [HD, s]
    cos = np.cos(phase).astype(np.float32)
    sin = np.sin(phase).astype(np.float32)
    sin[: HD // 2] *= -1.0
    return np.ascontiguousarray(cos), np.ascontiguousarray(sin)


def build_program(s=S, dm=DM):
    """Build the per-core Bass program. s, dm scaled down for sim tests."""
    W = s // 4                      # span width (512 full-size)
    NSP = 4                         # spans
    NKC = dm // P                   # contraction chunks (32)
    KBS = W // P                    # k-blocks per span step (4)
    NTB = s // P                    # t-blocks (16)
    dq, dkv = DQ, DKV
    NH = HEADS_PER_CORE

    nc = bacc.Bacc(None, target_bir_lowering=False, debug=True)

    xt_q = nc.dram_tensor("xt_q", [dm, s], F32R, kind="ExternalInput")
    xt_k = nc.dram_tensor("xt_k", [dm, s], F32R, kind="ExternalInput")
    xt_v = nc.dram_tensor("xt_v", [dm, s], F32R, kind="ExternalInput")
    wq = nc.dram_tensor("wq", [dm, dq], F32R, kind="ExternalInput")
    wk = nc.dram_tensor("wk", [dm, dkv], F32R, kind="ExternalInput")
    wv = nc.dram_tensor("wv", [dm, dkv], F32R, kind="ExternalInput")
    wo = nc.dram_tensor("wo", [dq, dm], F32R, kind="ExternalInput")
    cos_d = nc.dram_tensor("cos_t", [P, s], F32, kind="ExternalInput")
    sin_d = nc.dram_tensor("sin_t", [P, s], F32, kind="ExternalInput")
    mask_d = nc.dram_tensor("mask01", [P, KBS * W], F32, kind="ExternalInput")
    ones_d = nc.dram_tensor("ones", [P, 1], F32R, kind="ExternalInput")

    out_t = nc.dram_tensor("out_t", [dm, s], F32, kind="ExternalOutput")
    kht_out = nc.dram_tensor("kht_out", [dkv, s], F32, kind="ExternalOutput")
    vh_out = nc.dram_tensor("vh_out", [s, dkv], F32, kind="ExternalOutput")

    qt_scr = nc.dram_tensor("qt_scr", [dq, s], F32R)
    ot_scr = nc.dram_tensor("ot_scr", [dq, s], F32R)

    with tile.TileContext(nc) as tc:
        # ---- persistent tiles (live through phases A+B) ----
        with tc.tile_pool(name="persist", bufs=1) as persist:
            kht_sb = persist.tile([P, KV_PER_CORE, s], F32R, name="kht_sb")
            vh_sb = persist.tile([P, NTB, dkv], F32R, name="vh_sb")
            cos_sb = persist.tile([P, s], F32, name="cos_sb")
            sin_sb = persist.tile([P, s], F32, name="sin_sb")
            ones_sb = persist.tile([P, 1], F32R, name="ones_sb")
            nc.sync.dma_start(out=cos_sb, in_=cos_d[:, :])
            nc.sync.dma_start(out=sin_sb, in_=sin_d[:, :])
            nc.sync.dma_start(out=ones_sb, in_=ones_d[:, :])

            def rope_evict(pool, psum_t, sp, f32r_out, f32_out=None):
                """softcap-free RoPE eviction of a [P, W] psum tile at span sp.

                f32r_out: f32r SBUF destination AP.  f32_out: optional extra
                full-f32 SBUF destination (for the k-cache).
                """
                cs = cos_sb[:, sp * W:(sp + 1) * W]
                sn = sin_sb[:, sp * W:(sp + 1) * W]
                s_t = pool.tile([P, W], F32, name="rope_s")
                nc.scalar.copy(out=s_t, in_=psum_t)
                rot = pool.tile([P, W], F32, name="rope_rot")
                h = HD // 2
                nc.vector.tensor_copy(out=rot[0:h, :], in_=s_t[h:P, :])
                nc.vector.tensor_copy(out=rot[h:P, :], in_=s_t[0:h, :])
                nc.vector.tensor_tensor(s_t, s_t, cs, alu.AluOpType.mult)
                nc.vector.tensor_tensor(rot, rot, sn, alu.AluOpType.mult)
                if f32_out is not None:
                    nc.vector.tensor_tensor(f32_out, s_t, rot, alu.AluOpType.add)
                    nc.vector.tensor_copy(out=f32r_out, in_=f32_out)
                else:
                    nc.vector.tensor_tensor(f32r_out, s_t, rot, alu.AluOpType.add)

            # ================= Phase A1: K projection + RoPE ==============
            with tc.tile_pool(name="a1", bufs=1) as a1, \
                 tc.tile_pool(name="a1x", bufs=3) as a1x, \
                 tc.tile_pool(name="a1e", bufs=2) as a1e, \
                 tc.tile_pool(name="a1p", bufs=1, space="PSUM") as a1p:
                wk_sb = a1.tile([P, NKC, dkv], F32R, name="wk_sb")
                nc.sync.dma_start(
                    out=wk_sb, in_=wk[:, :].rearrange("(n p) m -> p n m", p=P))
                kpsum = [
                    a1p.tile([P, W], F32, name=f"kpsum{i}")
                    for i in range(KV_PER_CORE * NSP)
                ]
                for kc in range(NKC):
                    xtk_t = a1x.tile([P, s], F32R, name="xtk_t")
                    nc.sync.dma_start(out=xtk_t, in_=xt_k[kc * P:(kc + 1) * P, :])
                    for dk in range(KV_PER_CORE):
                        for sp in range(NSP):
                            nc.tensor.matmul(
                                kpsum[dk * NSP + sp],
                                lhsT=wk_sb[:, kc, dk * HD:(dk + 1) * HD],
                                rhs=xtk_t[:, sp * W:(sp + 1) * W],
                                start=(kc == 0), stop=(kc == NKC - 1))
                for dk in range(KV_PER_CORE):
                    for sp in range(NSP):
                        kc_f32 = a1e.tile([P, W], F32, name="kc_f32")
                        rope_evict(a1e, kpsum[dk * NSP + sp], sp,
                                   kht_sb[:, dk, sp * W:(sp + 1) * W], kc_f32)
                        nc.sync.dma_start(
                            out=kht_out[dk * HD:(dk + 1) * HD, sp * W:(sp + 1) * W],
                            in_=kc_f32)

            # ================= Phase A2: V projection =====================
            with tc.tile_pool(name="a2", bufs=1) as a2, \
                 tc.tile_pool(name="a2x", bufs=3) as a2x, \
                 tc.tile_pool(name="a2e", bufs=3) as a2e, \
                 tc.tile_pool(name="a2p", bufs=1, space="PSUM") as a2p:
                wv_sb = a2.tile([P, NKC, dkv], F32R, name="wv_sb")
                nc.sync.dma_start(
                    out=wv_sb, in_=wv[:, :].rearrange("(n p) m -> p n m", p=P))
                TBG = NTB // 2
                for g in range(2):
                    vpsum = [a2p.tile([P, dkv], F32, name=f"vpsum{i}")
                             for i in range(TBG)]
                    for kc in range(NKC):
                        xtv_t = a2x.tile([P, TBG * P], F32R, name="xtv_t")
                        nc.sync.dma_start(
                            out=xtv_t,
                            in_=xt_v[kc * P:(kc + 1) * P,
                                     g * TBG * P:(g + 1) * TBG * P])
                        for tb in range(TBG):
                            nc.tensor.matmul(
                                vpsum[tb],
                                lhsT=xtv_t[:, tb * P:(tb + 1) * P],
                                rhs=wv_sb[:, kc, :],
                                start=(kc == 0), stop=(kc == NKC - 1))
                    for tb in range(TBG):
                        tbg = g * TBG + tb
                        nc.scalar.copy(out=vh_sb[:, tbg, :], in_=vpsum[tb])
                        vc_f32 = a2e.tile([P, dkv], F32, name="vc_f32")
                        nc.scalar.copy(out=vc_f32, in_=vpsum[tb])
                        nc.sync.dma_start(
                            out=vh_out[tbg * P:(tbg + 1) * P, :], in_=vc_f32)

            # ================= Phase A3: Q projection + RoPE -> DRAM ======
            with tc.tile_pool(name="a3", bufs=1) as a3, \
                 tc.tile_pool(name="a3x", bufs=4) as a3x, \
                 tc.tile_pool(name="a3e", bufs=2) as a3e, \
                 tc.tile_pool(name="a3p", bufs=1, space="PSUM") as a3p:
                wq_sb = a3.tile([P, NKC, dq], F32R, name="wq_sb")
                nc.sync.dma_start(
                    out=wq_sb, in_=wq[:, :].rearrange("(n p) m -> p n m", p=P))
                for sp in range(NSP):
                    qpsum = [a3p.tile([P, W], F32, name=f"qpsum{i}")
                             for i in range(NH)]
                    for kc in range(NKC):
                        xtq_t = a3x.tile([P, W], F32R, name="xtq_t")
                        nc.sync.dma_start(
                            out=xtq_t,
                            in_=xt_q[kc * P:(kc + 1) * P, sp * W:(sp + 1) * W])
                        for dqb in range(NH):
                            nc.tensor.matmul(
                                qpsum[dqb],
                                lhsT=wq_sb[:, kc, dqb * HD:(dqb + 1) * HD],
                                rhs=xtq_t,
                                start=(kc == 0), stop=(kc == NKC - 1))
                    for dqb in range(NH):
                        q_f32r = a3e.tile([P, W], F32R, name="q_f32r")
                        rope_evict(a3e, qpsum[dqb], sp, q_f32r)
                        nc.sync.dma_start(
                            out=qt_scr[dqb * HD:(dqb + 1) * HD,
                                       sp * W:(sp + 1) * W],
                            in_=q_f32r)

            # ================= Phase B: attention =========================
            with tc.tile_pool(name="bmask", bufs=1) as bmask, \
                 tc.tile_pool(name="bq", bufs=2) as bq, \
                 tc.tile_pool(name="bo", bufs=2) as bo, \
                 tc.tile_pool(name="bw", bufs=3) as bw, \
                 tc.tile_pool(name="bacc", bufs=2) as baccp, \
                 tc.tile_pool(name="bn", bufs=2) as bn, \
                 tc.tile_pool(name="bsp", bufs=3, space="PSUM") as bsp, \
                 tc.tile_pool(name="bop", bufs=2, space="PSUM") as bop, \
                 tc.tile_pool(name="brp", bufs=2, space="PSUM") as brp:
                mask_sb = bmask.tile([P, KBS, W], F32, name="mask_sb")
                nc.sync.dma_start(
                    out=mask_sb,
                    in_=mask_d[:, :].rearrange("p (r w) -> p r w", r=KBS))
                qt_r = qt_scr[:, :].rearrange("(h p) t -> p h t", p=HD)
                ot_r = ot_scr[:, :].rearrange("(h p) t -> p h t", p=HD)
                for sp in range(NSP):
                    qt_sb = bq.tile([HD, NH, W], F32R, name="qt_sb")
                    nc.sync.dma_start(
                        out=qt_sb, in_=qt_r[:, :, sp * W:(sp + 1) * W])
                    ot_sb = bo.tile([HD, NH, W], F32R, name="ot_sb")
                    kbmax = (sp + 1) * KBS
                    for h in range(NH):
                        kv = h // GROUP
                        opsum = bop.tile([HD, W], F32, name="opsum")
                        acc = baccp.tile([P, W], F32R, name="acc")
                        for kb in range(kbmax):
                            spsum = bsp.tile([P, W], F32, name="spsum")
                            nc.tensor.matmul(
                                spsum,
                                lhsT=kht_sb[:, kv, kb * P:(kb + 1) * P],
                                rhs=qt_sb[:, h, :],
                                start=True, stop=True)
                            tanh_t = bw.tile([P, W], F32, name="tanh_t")
                            nc.scalar.activation(
                                tanh_t, spsum, AF.Tanh,
                                scale=float(ATTN_MULT / MAX_ATTN))
                            p_t = bw.tile([P, W], F32R, name="p_t")
                            nc.scalar.activation(
                                p_t, tanh_t, AF.Exp, scale=float(MAX_ATTN))
                            r = kb - sp * KBS
                            if r >= 0:
                                nc.vector.tensor_tensor(
                                    p_t, p_t, mask_sb[:, r, :],
                                    alu.AluOpType.mult)
                            nc.tensor.matmul(
                                opsum,
                                lhsT=vh_sb[:, kb, kv * HD:(kv + 1) * HD],
                                rhs=p_t,
                                start=(kb == 0), stop=(kb == kbmax - 1))
                            if kb == 0:
                                nc.vector.tensor_copy(out=acc, in_=p_t)
                            else:
                                nc.vector.tensor_tensor(
                                    acc, acc, p_t, alu.AluOpType.add)
                        rpsum = brp.tile([1, W], F32, name="rpsum")
                        nc.tensor.matmul(rpsum, lhsT=ones_sb, rhs=acc,
                                         start=True, stop=True)
                        recip = bn.tile([1, W], F32, name="recip")
                        nc.vector.reciprocal(recip, rpsum)
                        rb = bn.tile([P, W], F32, name="rb")
                        nc.gpsimd.partition_broadcast(rb, recip)
                        nc.vector.tensor_tensor(
                            ot_sb[:, h, :], opsum, rb, alu.AluOpType.mult)
                    nc.sync.dma_start(
                        out=ot_r[:, :, sp * W:(sp + 1) * W], in_=ot_sb)

        # ================= Phase C: output projection =====================
        with tc.tile_pool(name="c1", bufs=1) as c1, \
             tc.tile_pool(name="cx", bufs=2) as cx, \
             tc.tile_pool(name="ce", bufs=3) as ce, \
             tc.tile_pool(name="cp", bufs=4, space="PSUM") as cp:
            wo_sb = c1.tile([P, NH, dm], F32R, name="wo_sb")
            nc.sync.dma_start(
                out=wo_sb, in_=wo[:, :].rearrange("(n p) m -> p n m", p=P))
            ot_r2 = ot_scr[:, :].rearrange("(h p) t -> p h t", p=HD)
            for sp in range(NSP):
                otc_sb = cx.tile([HD, NH, W], F32R, name="otc_sb")
                nc.sync.dma_start(
                    out=otc_sb, in_=ot_r2[:, :, sp * W:(sp + 1) * W])
                for dmb in range(dm // P):
                    cpsum = cp.tile([P, W], F32, name="cpsum")
                    for qc in range(NH):
                        nc.tensor.matmul(
                            cpsum,
                            lhsT=wo_sb[:, qc, dmb * P:(dmb + 1) * P],
                            rhs=otc_sb[:, qc, :],
                            start=(qc == 0), stop=(qc == NH - 1))
                    cout = ce.tile([P, W], F32, name="cout")
                    nc.scalar.copy(out=cout, in_=cpsum)
                    nc.sync.dma_start(
                        out=out_t[dmb * P:(dmb + 1) * P, sp * W:(sp + 1) * W],
                        in_=cout)

    nc.compile()
    return nc


def _prep_inputs(query, key, value, Wq, Wk, Wv, Wo, s, dm):
    """Build the 8 per-core input maps from the full tensors."""
    W = s // 4
    KBS = W // P
    cos, sin = _rope_tables(s)
    mask01 = np.zeros((P, KBS * W), dtype=np.float32)
    for r in range(KBS):
        k_idx = np.arange(P)[:, None] + r * P
        q_idx = np.arange(W)[None, :]
        mask01[:, r * W:(r + 1) * W] = (k_idx <= q_idx).astype(np.float32)
    ones = np.ones((P, 1), dtype=np.float32)

    xt = []
    for b in range(B):
        xt.append((
            np.ascontiguousarray(query[b].T),
            np.ascontiguousarray(key[b].T),
            np.ascontiguousarray(value[b].T),
        ))
    in_maps = []
    for c in range(NCORES):
        b, j = divmod(c, NCORES // B)
        xq, xk, xv = xt[b]
        in_maps.append({
            "xt_q": xq, "xt_k": xk, "xt_v": xv,
            "wq": np.ascontiguousarray(Wq[:, j * DQ:(j + 1) * DQ]),
            "wk": np.ascontiguousarray(Wk[:, j * DKV:(j + 1) * DKV]),
            "wv": np.ascontiguousarray(Wv[:, j * DKV:(j + 1) * DKV]),
            "wo": np.ascontiguousarray(Wo[j * DQ:(j + 1) * DQ, :]),
            "cos_t": cos, "sin_t": sin, "mask01": mask01, "ones": ones,
        })
    return in_maps


def kernel(query, key, value, mask, Wq, Wk, Wv, Wo, trace=False):
    """Full-size entry point: full inputs in, full outputs out."""
    query = np.asarray(query, dtype=np.float32)
    key = np.asarray(key, dtype=np.float32)
    value = np.asarray(value, dtype=np.float32)
    Wq = np.asarray(Wq, dtype=np.float32)
    Wk = np.asarray(Wk, dtype=np.float32)
    Wv = np.asarray(Wv, dtype=np.float32)
    Wo = np.asarray(Wo, dtype=np.float32)
    # mask is causal by construction (tril); the kernel exploits it directly.

    nc = build_program(S, DM)
    in_maps = _prep_inputs(query, key, value, Wq, Wk, Wv, Wo, S, DM)
    res = run_bass_kernel_spmd(nc, in_maps, list(range(NCORES)), trace=trace)

    out = np.zeros((B, S, DM), dtype=np.float32)
    cache_k = np.zeros((B, NKV, S, HD), dtype=np.float32)
    cache_v = np.zeros((B, NKV, S, HD), dtype=np.float32)
    JP = NCORES // B
    for c in range(NCORES):
        b, j = divmod(c, JP)
        r = res.results[c]
        out[b] += r["out_t"].T
        kht = r["kht_out"]              # [DKV, S]
        vh = r["vh_out"]                # [S, DKV]
        for i in range(KV_PER_CORE):
            cache_k[b, KV_PER_CORE * j + i] = kht[i * HD:(i + 1) * HD, :].T
            cache_v[b, KV_PER_CORE * j + i] = vh[:, i * HD:(i + 1) * HD]
    if trace:
        kernel._last_exec_time_ns = res.exec_time_ns
    return out, cache_k, cache_v


# revision 8
# speedup vs baseline: 1.0273x; 1.0273x over previous
"""Trainium2 Bass kernel for CachingMultiHeadAttention (GQA + RoPE + softcap).

Reference semantics (B=2, S=2048, D=4096, 32 q-heads, 8 kv-heads, hd=128):
    qh = rope(x_q @ Wq); kh = rope(x_k @ Wk); vh = x_v @ Wv
    logits = softcap_30(qh kh^T / sqrt(128)) causal-masked
    out = softmax(logits) vh @ Wo ; also returns cache_k, cache_v

Sharding: 8 cores = (2 batches) x (4 kv-head pairs). Each core computes its
batch's projections for its 2 kv-heads / 8 q-heads against full D_MODEL,
attention, and a partial out^T (summed on host over the 4 cores per batch).

All device matmuls run in float32r (fast fp32 mode, 1 cyc/row at N>=256).
Activations are fed pre-transposed ([D, S]) from the host so every matmul
contracts over the partition dim without on-device transposes.

Pipeline (phases overlap via per-span DRAM staging + Tile dataflow):
  A1 K proj (+RoPE) -> kht_sb resident      [2 waves, ping-pong PSUM]
  A2 V proj -> vh_sb resident               [2 groups, ping-pong PSUM]
  A3 Q proj (+RoPE) -> qt_scr[span] DRAM    [per span; overlaps B]
  B  attention per span -> ot_scr[span]     [softcap=ACT tanh/exp pairs,
                                             additive -100 mask pre-exp,
                                             rowsum = ones-matmul over DVE-
                                             accumulated P, approx-recip +
                                             gpsimd partition_broadcast]
  C  out proj, 2 half phases (spans 01/23)  [overlaps B spans 2-3]
"""
import sys
sys.path.insert(0, "/opt/trn_rl_repo")

import numpy as np

import concourse.bass as bass
from concourse import bacc
import concourse.mybir as mybir
import concourse.tile as tile
import concourse.alu_op_type as alu
from concourse.bass_utils import run_bass_kernel_spmd

F32 = mybir.dt.float32
F32R = mybir.dt.float32r
AF = mybir.ActivationFunctionType
MUL = alu.AluOpType.mult
ADD = alu.AluOpType.add

# Problem constants
B, S, DM = 2, 2048, 4096
NQ, NKV, HD = 32, 8, 128
GROUP = NQ // NKV
ATTN_MULT = 1.0 / np.sqrt(128.0)
MAX_ATTN = 30.0
ROPE_BASE = 10000.0
MASK_NEG = -100.0            # additive pre-exp mask value (exp(30*-100) == 0)

P = 128
NCORES = 8
HEADS_PER_CORE = NQ // (NCORES // B)      # 8
KV_PER_CORE = NKV // (NCORES // B)        # 2
DQ = HEADS_PER_CORE * HD                  # 1024
DKV = KV_PER_CORE * HD                    # 256


def _rope_tables(s):
    """cos table and sign-baked sin table, [HD, s] f32.

    rope(x)[d,t] = x[d,t]*cos[d,t] + x[(d+64)%128, t]*sin_sgn[d,t]
    with sin_sgn negative for d < 64 (matches -x2 in the reference).
    """
    j = np.arange(HD // 2, dtype=np.float64)
    inv_freq = ROPE_BASE ** (-2.0 * j / HD)
    t = np.arange(s, dtype=np.float64)
    phase = np.concatenate([inv_freq, inv_freq])[:, None] * t[None, :]
    cos = np.cos(phase).astype(np.float32)
    sin = np.sin(phase).astype(np.float32)
    sin[: HD // 2] *= -1.0
    return np.ascontiguousarray(cos), np.ascontiguousarray(sin)


def build_program(s=S, dm=DM):
    """Build the per-core Bass program. s, dm scaled down for sim tests."""
    W = s // 4                      # span width (512 full-size)
    NSP = 4
    NKC = dm // P                   # contraction chunks (32)
    KBS = W // P                    # k-blocks per span step (4)
    NTB = s // P                    # t-blocks (16)
    dq, dkv = DQ, DKV
    NH = HEADS_PER_CORE

    nc = bacc.Bacc(None, target_bir_lowering=False, debug=True)

    xt_q = nc.dram_tensor("xt_q", [dm, s], F32R, kind="ExternalInput")
    xt_k = nc.dram_tensor("xt_k", [dm, s], F32R, kind="ExternalInput")
    xt_v = nc.dram_tensor("xt_v", [dm, s], F32R, kind="ExternalInput")
    wq = nc.dram_tensor("wq", [dm, dq], F32R, kind="ExternalInput")
    wk = nc.dram_tensor("wk", [dm, dkv], F32R, kind="ExternalInput")
    wv = nc.dram_tensor("wv", [dm, dkv], F32R, kind="ExternalInput")
    wo = nc.dram_tensor("wo", [dq, dm], F32R, kind="ExternalInput")
    cos_d = nc.dram_tensor("cos_t", [P, s], F32, kind="ExternalInput")
    sin_d = nc.dram_tensor("sin_t", [P, s], F32, kind="ExternalInput")
    mask_d = nc.dram_tensor("maskneg", [P, KBS * W], F32, kind="ExternalInput")
    ones_d = nc.dram_tensor("ones", [P, 1], F32R, kind="ExternalInput")

    out_t = nc.dram_tensor("out_t", [dm, s], F32, kind="ExternalOutput")
    kht_out = nc.dram_tensor("kht_out", [dkv, s], F32, kind="ExternalOutput")
    vh_out = nc.dram_tensor("vh_out", [s, dkv], F32, kind="ExternalOutput")

    # per-span staging (separate tensors => fine-grained cross-phase deps)
    qt_scr = [nc.dram_tensor(f"qt_scr{i}", [dq, W], F32R) for i in range(NSP)]
    ot_scr = [nc.dram_tensor(f"ot_scr{i}", [dq, W], F32R) for i in range(NSP)]

    with tile.TileContext(nc) as tc:
        with tc.tile_pool(name="persist", bufs=1) as persist:
            kht_sb = persist.tile([P, KV_PER_CORE, s], F32R, name="kht_sb")
            vh_sb = persist.tile([P, NTB, dkv], F32R, name="vh_sb")
            cos_sb = persist.tile([P, s], F32, name="cos_sb")
            sin_sb = persist.tile([P, s], F32, name="sin_sb")
            ones_sb = persist.tile([P, 1], F32R, name="ones_sb")
            nc.sync.dma_start(out=cos_sb, in_=cos_d[:, :])
            nc.sync.dma_start(out=sin_sb, in_=sin_d[:, :])
            nc.sync.dma_start(out=ones_sb, in_=ones_d[:, :])

            def rope_evict(pool, psum_t, t0, w, f32r_out, f32_out=None):
                """RoPE a [P, w] psum tile covering positions [t0, t0+w)."""
                cs = cos_sb[:, t0:t0 + w]
                sn = sin_sb[:, t0:t0 + w]
                s_t = pool.tile([P, W], F32, name="rope_s")[:, :w]
                nc.scalar.copy(out=s_t, in_=psum_t)
                rot = pool.tile([P, W], F32, name="rope_rot")[:, :w]
                h = HD // 2
                nc.vector.tensor_copy(out=rot[0:h, :], in_=s_t[h:P, :])
                nc.vector.tensor_copy(out=rot[h:P, :], in_=s_t[0:h, :])
                nc.vector.tensor_tensor(s_t, s_t, cs, MUL)
                nc.vector.tensor_tensor(rot, rot, sn, MUL)
                if f32_out is not None:
                    nc.vector.tensor_tensor(f32_out, s_t, rot, ADD)
                    nc.vector.tensor_copy(out=f32r_out, in_=f32_out)
                else:
                    nc.vector.tensor_tensor(f32r_out, s_t, rot, ADD)

            # ================= Phase A1: K projection + RoPE ==============
            with tc.tile_pool(name="a1", bufs=1) as a1, \
                 tc.tile_pool(name="a1x", bufs=3) as a1x, \
                 tc.tile_pool(name="a1e", bufs=2) as a1e, \
                 tc.tile_pool(name="a1p", bufs=1, space="PSUM") as a1p:
                wk_sb = a1.tile([P, NKC, dkv], F32R, name="wk_sb")
                nc.sync.dma_start(
                    out=wk_sb, in_=wk[:, :].rearrange("(n p) m -> p n m", p=P))
                for wv_ in range(2):          # wave = span pair
                    kpsum = [a1p.tile([P, W], F32, name=f"kpsum{wv_}_{i}")
                             for i in range(KV_PER_CORE * 2)]
                    for kc in range(NKC):
                        xtk_t = a1x.tile([P, 2 * W], F32R, name="xtk_t")
                        nc.sync.dma_start(
                            out=xtk_t,
                            in_=xt_k[kc * P:(kc + 1) * P,
                                     wv_ * 2 * W:(wv_ + 1) * 2 * W])
                        for dk in range(KV_PER_CORE):
                            for s2 in range(2):
                                nc.tensor.matmul(
                                    kpsum[dk * 2 + s2],
                                    lhsT=wk_sb[:, kc, dk * HD:(dk + 1) * HD],
                                    rhs=xtk_t[:, s2 * W:(s2 + 1) * W],
                                    start=(kc == 0), stop=(kc == NKC - 1))
                    for dk in range(KV_PER_CORE):
                        for s2 in range(2):
                            sp = wv_ * 2 + s2
                            kc_f32 = a1e.tile([P, W], F32, name="kc_f32")
                            rope_evict(a1e, kpsum[dk * 2 + s2], sp * W, W,
                                       kht_sb[:, dk, sp * W:(sp + 1) * W],
                                       kc_f32)
                            nc.sync.dma_start(
                                out=kht_out[dk * HD:(dk + 1) * HD,
                                            sp * W:(sp + 1) * W],
                                in_=kc_f32)

            # ================= Phase A2: V projection =====================
            with tc.tile_pool(name="a2", bufs=1) as a2, \
                 tc.tile_pool(name="a2x", bufs=3) as a2x, \
                 tc.tile_pool(name="a2e", bufs=3) as a2e, \
                 tc.tile_pool(name="a2p", bufs=1, space="PSUM") as a2p:
                wv_sb = a2.tile([P, NKC, dkv], F32R, name="wv_sb")
                nc.sync.dma_start(
                    out=wv_sb, in_=wv[:, :].rearrange("(n p) m -> p n m", p=P))
                TBG = NTB // 2
                for g in range(2):
                    vpsum = [a2p.tile([P, dkv], F32, name=f"vpsum{i}")
                             for i in range(TBG)]
                    for kc in range(NKC):
                        xtv_t = a2x.tile([P, TBG * P], F32R, name="xtv_t")
                        nc.sync.dma_start(
                            out=xtv_t,
                            in_=xt_v[kc * P:(kc + 1) * P,
                                     g * TBG * P:(g + 1) * TBG * P])
                        for tb in range(TBG):
                            nc.tensor.matmul(
                                vpsum[tb],
                                lhsT=xtv_t[:, tb * P:(tb + 1) * P],
                                rhs=wv_sb[:, kc, :],
                                start=(kc == 0), stop=(kc == NKC - 1))
                    for tb in range(TBG):
                        tbg = g * TBG + tb
                        nc.vector.tensor_copy(
                            out=vh_sb[:, tbg, :], in_=vpsum[tb])
                        vc_f32 = a2e.tile([P, dkv], F32, name="vc_f32")
                        nc.scalar.copy(out=vc_f32, in_=vpsum[tb])
                        nc.sync.dma_start(
                            out=vh_out[tbg * P:(tbg + 1) * P, :], in_=vc_f32)

            # ================= Phase A3: Q projection + RoPE -> DRAM ======
            with tc.tile_pool(name="a3", bufs=1) as a3, \
                 tc.tile_pool(name="a3x", bufs=4) as a3x, \
                 tc.tile_pool(name="a3e", bufs=2) as a3e, \
                 tc.tile_pool(name="a3p", bufs=1, space="PSUM") as a3p:
                wq_sb = a3.tile([P, NKC, dq], F32R, name="wq_sb")
                nc.sync.dma_start(
                    out=wq_sb, in_=wq[:, :].rearrange("(n p) m -> p n m", p=P))
                for sp in range(NSP):
                    qpsum = [a3p.tile([P, W], F32, name=f"qpsum{i}")
                             for i in range(NH)]
                    for kc in range(NKC):
                        xtq_t = a3x.tile([P, W], F32R, name="xtq_t")
                        nc.sync.dma_start(
                            out=xtq_t,
                            in_=xt_q[kc * P:(kc + 1) * P, sp * W:(sp + 1) * W])
                        for dqb in range(NH):
                            nc.tensor.matmul(
                                qpsum[dqb],
                                lhsT=wq_sb[:, kc, dqb * HD:(dqb + 1) * HD],
                                rhs=xtq_t,
                                start=(kc == 0), stop=(kc == NKC - 1))
                    for dqb in range(NH):
                        q_f32r = a3e.tile([P, W], F32R, name="q_f32r")
                        rope_evict(a3e, qpsum[dqb], sp * W, W, q_f32r)
                        nc.sync.dma_start(
                            out=qt_scr[sp][dqb * HD:(dqb + 1) * HD, :],
                            in_=q_f32r)

            # ====== Phases B (attention) + C (out proj), interleaved ======
            with tc.tile_pool(name="bmask", bufs=1) as bmask, \
                 tc.tile_pool(name="bq", bufs=2) as bq, \
                 tc.tile_pool(name="bo", bufs=2) as bo, \
                 tc.tile_pool(name="bw", bufs=3) as bw, \
                 tc.tile_pool(name="bacc", bufs=2) as baccp, \
                 tc.tile_pool(name="bn", bufs=2) as bn, \
                 tc.tile_pool(name="bsp", bufs=1, space="PSUM") as bsp, \
                 tc.tile_pool(name="bop", bufs=2, space="PSUM") as bop, \
                 tc.tile_pool(name="brp", bufs=1, space="PSUM") as brp, \
                 tc.tile_pool(name="cot", bufs=1) as cot, \
                 tc.tile_pool(name="cw", bufs=3) as cw, \
                 tc.tile_pool(name="ce", bufs=3) as ce, \
                 tc.tile_pool(name="cp", bufs=3, space="PSUM") as cp:
                mask_sb = bmask.tile([P, KBS, W], F32, name="mask_sb")
                nc.sync.dma_start(
                    out=mask_sb,
                    in_=mask_d[:, :].rearrange("p (r w) -> p r w", r=KBS))

                def attention_span(sp):
                    qt_sb = bq.tile([HD, NH, W], F32R, name="qt_sb")
                    nc.sync.dma_start(
                        out=qt_sb,
                        in_=qt_scr[sp][:, :].rearrange("(h p) t -> p h t", p=HD))
                    ot_sb = bo.tile([HD, NH, W], F32R, name="ot_sb")
                    kbmax = (sp + 1) * KBS
                    for h in range(NH):
                        kv = h // GROUP
                        opsum = bop.tile([HD, W], F32, name="opsum")
                        acc = baccp.tile([P, W], F32R, name="acc")
                        for kb0 in range(0, kbmax, 2):
                            nk = min(2, kbmax - kb0)
                            spsum = bsp.tile([P, 2, W], F32,
                                             name="spsum")[:, :nk, :]
                            for i in range(nk):
                                nc.tensor.matmul(
                                    spsum[:, i, :],
                                    lhsT=kht_sb[:, kv,
                                                (kb0 + i) * P:(kb0 + i + 1) * P],
                                    rhs=qt_sb[:, h, :],
                                    start=True, stop=True)
                            tanh_t = bw.tile([P, 2, W], F32,
                                             name="tanh_t")[:, :nk, :]
                            nc.scalar.activation(
                                tanh_t, spsum, AF.Tanh,
                                scale=float(ATTN_MULT / MAX_ATTN))
                            r0 = kb0 - sp * KBS
                            if r0 + nk > 0:
                                rs = max(r0, 0)
                                o = rs - r0
                                nc.vector.tensor_tensor(
                                    tanh_t[:, o:nk, :], tanh_t[:, o:nk, :],
                                    mask_sb[:, rs:rs + nk - o, :], ADD)
                            p_t = bw.tile([P, 2, W], F32R,
                                          name="p_t")[:, :nk, :]
                            nc.scalar.activation(
                                p_t, tanh_t, AF.Exp, scale=float(MAX_ATTN))
                            for i in range(nk):
                                nc.tensor.matmul(
                                    opsum,
                                    lhsT=vh_sb[:, kb0 + i,
                                               kv * HD:(kv + 1) * HD],
                                    rhs=p_t[:, i, :],
                                    start=(kb0 + i == 0),
                                    stop=(kb0 + i == kbmax - 1))
                            for i in range(nk):
                                if kb0 + i == 0:
                                    nc.vector.tensor_copy(
                                        out=acc, in_=p_t[:, 0, :])
                                else:
                                    nc.vector.tensor_tensor(
                                        acc, acc, p_t[:, i, :], ADD)
                        rpsum = brp.tile([1, W], F32, name="rpsum")
                        nc.tensor.matmul(rpsum, lhsT=ones_sb, rhs=acc,
                                         start=True, stop=True)
                        recip = bn.tile([1, W], F32, name="recip")
                        nc.vector.reciprocal_approx_fast(recip, rpsum)
                        rb = bn.tile([P, W], F32, name="rb")
                        nc.gpsimd.partition_broadcast(rb, recip)
                        nc.vector.tensor_tensor(
                            ot_sb[:, h, :], opsum, rb, MUL)
                    nc.sync.dma_start(
                        out=ot_scr[sp][:, :].rearrange("(h p) t -> p h t", p=HD),
                        in_=ot_sb)

                def outproj_half(half):
                    otc = cot.tile([HD, 2, NH, W], F32R, name="otc")
                    for s2 in range(2):
                        sp = half * 2 + s2
                        nc.sync.dma_start(
                            out=otc[:, s2, :, :],
                            in_=ot_scr[sp][:, :].rearrange(
                                "(h p) t -> p h t", p=HD))
                    wo_r = wo[:, :].rearrange("(n p) m -> p n m", p=P)
                    for dmb in range(dm // P):
                        wo_t = cw.tile([P, NH, P], F32R, name="wo_t")
                        nc.sync.dma_start(
                            out=wo_t, in_=wo_r[:, :, dmb * P:(dmb + 1) * P])
                        for s2 in range(2):
                            sp = half * 2 + s2
                            cpsum = cp.tile([P, W], F32, name="cpsum")
                            for qc in range(NH):
                                nc.tensor.matmul(
                                    cpsum,
                                    lhsT=wo_t[:, qc, :],
                                    rhs=otc[:, s2, qc, :],
                                    start=(qc == 0), stop=(qc == NH - 1))
                            cout = ce.tile([P, W], F32, name="cout")
                            nc.vector.tensor_copy(out=cout, in_=cpsum)
                            nc.sync.dma_start(
                                out=out_t[dmb * P:(dmb + 1) * P,
                                          sp * W:(sp + 1) * W],
                                in_=cout)

                attention_span(0)
                attention_span(1)
                outproj_half(0)
                attention_span(2)
                attention_span(3)
                outproj_half(1)

    nc.compile()
    return nc


def _prep_inputs(query, key, value, Wq, Wk, Wv, Wo, s, dm):
    """Build the 8 per-core input maps from the full tensors."""
    W = s // 4
    KBS = W // P
    cos, sin = _rope_tables(s)
    maskneg = np.zeros((P, KBS * W), dtype=np.float32)
    for r in range(KBS):
        k_idx = np.arange(P)[:, None] + r * P
        q_idx = np.arange(W)[None, :]
        maskneg[:, r * W:(r + 1) * W] = np.where(
            k_idx <= q_idx, 0.0, MASK_NEG).astype(np.float32)
    ones = np.ones((P, 1), dtype=np.float32)

    xt = []
    for b in range(B):
        xt.append((
            np.ascontiguousarray(query[b].T),
            np.ascontiguousarray(key[b].T),
            np.ascontiguousarray(value[b].T),
        ))
    in_maps = []
    for c in range(NCORES):
        b, j = divmod(c, NCORES // B)
        xq, xk, xv = xt[b]
        in_maps.append({
            "xt_q": xq, "xt_k": xk, "xt_v": xv,
            "wq": np.ascontiguousarray(Wq[:, j * DQ:(j + 1) * DQ]),
            "wk": np.ascontiguousarray(Wk[:, j * DKV:(j + 1) * DKV]),
            "wv": np.ascontiguousarray(Wv[:, j * DKV:(j + 1) * DKV]),
            "wo": np.ascontiguousarray(Wo[j * DQ:(j + 1) * DQ, :]),
            "cos_t": cos, "sin_t": sin, "maskneg": maskneg, "ones": ones,
        })
    return in_maps


def kernel(query, key, value, mask, Wq, Wk, Wv, Wo, trace=False):
    """Full-size entry point: full inputs in, full outputs out."""
    query = np.asarray(query, dtype=np.float32)
    key = np.asarray(key, dtype=np.float32)
    value = np.asarray(value, dtype=np.float32)
    Wq = np.asarray(Wq, dtype=np.float32)
    Wk = np.asarray(Wk, dtype=np.float32)
    Wv = np.asarray(Wv, dtype=np.float32)
    Wo = np.asarray(Wo, dtype=np.float32)
    # mask is causal by construction (tril); the kernel exploits it directly.

    nc = build_program(S, DM)
    in_maps = _prep_inputs(query, key, value, Wq, Wk, Wv, Wo, S, DM)
    res = run_bass_kernel_spmd(nc, in_maps, list(range(NCORES)), trace=trace)

    out = np.zeros((B, S, DM), dtype=np.float32)
    cache_k = np.zeros((B, NKV, S, HD), dtype=np.float32)
    cache_v = np.zeros((B, NKV, S, HD), dtype=np.float32)
    JP = NCORES // B
    for c in range(NCORES):
        b, j = divmod(c, JP)
        r = res.results[c]
        out[b] += r["out_t"].T
        kht = r["kht_out"]              # [DKV, S]
        vh = r["vh_out"]                # [S, DKV]
        for i in range(KV_PER_CORE):
            cache_k[b, KV_PER_CORE * j + i] = kht[i * HD:(i + 1) * HD, :].T
            cache_v[b, KV_PER_CORE * j + i] = vh[:, i * HD:(i + 1) * HD]
    if trace:
        kernel._last_exec_time_ns = res.exec_time_ns
    return out, cache_k, cache_v
